# revision 6
# baseline (speedup 1.0000x reference)
"""MoE (Gemma-style 8-expert top-2) Trainium2 kernel.

Strategy (expert-parallel over 8 NeuronCores):
  - Host: merge duplicate (token, expert) assignments, build per-expert token
    lists, gather+transpose x into xT_e [H, C] per expert (zero-padded to a
    common capacity C).  This is the "dispatch" all-to-all done host-side,
    which the full-input/full-output contract allows.
  - Device (per core e): dense expert MLP on its C tokens, all in transposed
    layout so every matmul uses natural weight layouts with zero on-device
    transposes:
        gateT = Wg_e.T-contract:  gateT[i, c] = sum_h Wg[h,i] * xT[h,c]
        upT   likewise
        hT    = gelu_tanh(gateT) * upT                    [I, C]
        yT[h, c] = sum_i Wd[i,h] * hT[i,c]                [H, C]
    Matmuls run as float32r (TF32-like, full PE rate at free-dim >= 256)
    with fp32 PSUM accumulation.
  - Host: combine — out[t] += route[t,e] * yT_e[:, pos].T  (the "combine"
    all-to-all), with route exactly matching the reference's scatter-add.
"""

import numpy as np

import concourse.bass as bass
import concourse.mybir as mybir
import concourse.tile as tile
from concourse import bacc


def _install_ntff_hook_shim():
    """The agent image's `antenv` lacks `axon_hooks`, which bass_utils
    imports unconditionally when tracing under axon.  Provide the module
    and register the ctypes-based NTFF profile hook so BASS_TRACE=1 yields
    real HW profiles.  Degrades silently if anything is missing."""
    import sys
    import types

    try:
        import antenv

        try:
            from antenv import axon_hooks  # noqa: F401

            return
        except ImportError:
            pass
        mod = types.ModuleType("antenv.axon_hooks")
        mod._hook = None
        mod.set_axon_ntff_profile_hook = lambda h: setattr(mod, "_hook", h)
        mod.get_axon_ntff_profile_hook = lambda: mod._hook
        sys.modules["antenv.axon_hooks"] = mod
        antenv.axon_hooks = mod
        import os

        so_path = "/opt/axon/libaxon_pjrt.so"
        if os.path.exists(so_path):
            from trn_agent_boot.trn_boot import _ntff_profile_via_ctypes

            mod._hook = _ntff_profile_via_ctypes(so_path)
    except Exception:
        pass


_install_ntff_hook_shim()

from concourse.bass_utils import run_bass_kernel_spmd

H = 2048
I = 4096
E = 8
P = 128
F32 = mybir.dt.float32
F32R = mybir.dt.float32r

# Results of the last device run (for test harnesses to inspect profiling).
LAST_RESULTS = None

_PROGRAM_CACHE: dict[int, "bass.Bass"] = {}


def _build_program(C: int) -> "bass.Bass":
    """Bass program for one core: expert MLP on C tokens (transposed layout)."""
    assert C % 32 == 0 and C >= 256 and C <= 512
    KH = H // P  # 16 contraction chunks for gate/up
    MI = I // P  # 32 output tiles of I
    KI = I // P  # 32 contraction chunks for down
    MH = H // P  # 16 output tiles of H

    nc = bacc.Bacc("TRN2", target_bir_lowering=False)

    xT = nc.dram_tensor("xT", [H, C], F32R, kind="ExternalInput")
    Wg = nc.dram_tensor("Wg", [H, I], F32R, kind="ExternalInput")
    Wu = nc.dram_tensor("Wu", [H, I], F32R, kind="ExternalInput")
    Wd = nc.dram_tensor("Wd", [I, H], F32R, kind="ExternalInput")
    yT = nc.dram_tensor("yT", [H, C], F32, kind="ExternalOutput")

    # DRAM views with the partition dim innermost of the row index:
    # row r = k*128 + p  ->  [p, k, cols]
    xT_r = xT.rearrange("(k p) c -> p k c", p=P)
    Wg_r = Wg.rearrange("(k p) i -> p k i", p=P)
    Wu_r = Wu.rearrange("(k p) i -> p k i", p=P)
    Wd_r = Wd.rearrange("(k p) h -> p k h", p=P)
    yT_r = yT.rearrange("(m p) c -> p m c", p=P)

    gelu = mybir.ActivationFunctionType.Gelu_apprx_tanh

    with tile.TileContext(nc) as tc:
        with (
            tc.tile_pool(name="xpool", bufs=1) as xpool,
            tc.tile_pool(name="hpool", bufs=1) as hpool,
            tc.tile_pool(name="wpool", bufs=2) as wpool,
            tc.tile_pool(name="tpool", bufs=3) as tpool,
            tc.tile_pool(name="psum", bufs=2, space="PSUM") as psum_pool,
        ):
            # x resident in SBUF: [128, 16, C] (32 KB/partition at C=512)
            xsb = xpool.tile([P, KH, C], F32R)
            for q in range(4):
                nc.sync.dma_start(
                    out=xsb[:, q * (KH // 4) : (q + 1) * (KH // 4), :],
                    in_=xT_r[:, q * (KH // 4) : (q + 1) * (KH // 4), :],
                )

            # h resident in SBUF: [128, 32, C] (64 KB/partition at C=512)
            hsb = hpool.tile([P, KI, C], F32R)

            # ---- Phase 1: gateT/upT -> hT, one I-tile (128 rows) at a time
            for m in range(MI):
                wg_t = wpool.tile([P, KH, P], F32R, tag="wg")
                wu_t = wpool.tile([P, KH, P], F32R, tag="wu")
                # split each weight-tile load in two so two DMA queues run
                # in parallel
                for hf in range(2):
                    ks = slice(hf * (KH // 2), (hf + 1) * (KH // 2))
                    nc.sync.dma_start(
                        out=wg_t[:, ks, :], in_=Wg_r[:, ks, m * P : (m + 1) * P]
                    )
                    nc.sync.dma_start(
                        out=wu_t[:, ks, :], in_=Wu_r[:, ks, m * P : (m + 1) * P]
                    )

                psum_g = psum_pool.tile([P, C], F32, tag="g")
                psum_u = psum_pool.tile([P, C], F32, tag="u")
                for k in range(KH):
                    nc.tensor.matmul(
                        psum_g,
                        wg_t[:, k, :],
                        xsb[:, k, :],
                        start=(k == 0),
                        stop=(k == KH - 1),
                    )
                for k in range(KH):
                    nc.tensor.matmul(
                        psum_u,
                        wu_t[:, k, :],
                        xsb[:, k, :],
                        start=(k == 0),
                        stop=(k == KH - 1),
                    )
                tg = tpool.tile([P, C], F32, tag="gelu")
                nc.scalar.activation(tg, psum_g, gelu)
                nc.vector.tensor_mul(hsb[:, m, :], tg, psum_u)

            # ---- Phase 2: downT -> yT, one H-tile (128 rows) at a time
            for m2 in range(MH):
                wd_t = wpool.tile([P, KI, P], F32R, tag="wd")
                for qf in range(2):
                    ks = slice(qf * (KI // 2), (qf + 1) * (KI // 2))
                    nc.sync.dma_start(
                        out=wd_t[:, ks, :], in_=Wd_r[:, ks, m2 * P : (m2 + 1) * P]
                    )
                psum_d = psum_pool.tile([P, C], F32, tag="d")
                for k2 in range(KI):
                    nc.tensor.matmul(
                        psum_d,
                        wd_t[:, k2, :],
                        hsb[:, k2, :],
                        start=(k2 == 0),
                        stop=(k2 == KI - 1),
                    )
                ysb = tpool.tile([P, C], F32, tag="y")
                nc.vector.tensor_copy(ysb, psum_d)
                nc.sync.dma_start(out=yT_r[:, m2, :], in_=ysb)

    nc.compile()
    return nc


def _get_program(C: int) -> "bass.Bass":
    if C not in _PROGRAM_CACHE:
        _PROGRAM_CACHE[C] = _build_program(C)
    return _PROGRAM_CACHE[C]


def kernel(x, selected_experts, routing_weights, Wg, Wu, Wd):
    global LAST_RESULTS
    x = np.ascontiguousarray(np.asarray(x), dtype=np.float32)
    se = np.asarray(selected_experts).astype(np.int64)
    rw = np.asarray(routing_weights).astype(np.float32)
    Wg = np.ascontiguousarray(np.asarray(Wg), dtype=np.float32)
    Wu = np.ascontiguousarray(np.asarray(Wu), dtype=np.float32)
    Wd = np.ascontiguousarray(np.asarray(Wd), dtype=np.float32)

    T, K = se.shape
    assert x.shape == (T, H) and Wg.shape == (E, H, I) and Wd.shape == (E, I, H)

    # Dense route matrix, identical to the reference's scatter-add (merges
    # duplicate expert picks within a token by summing their weights).
    flat_t = np.repeat(np.arange(T), K)
    flat_e = se.ravel()
    route = np.zeros((T, E), np.float32)
    np.add.at(route, (flat_t, flat_e), rw.ravel())
    present = np.zeros((T, E), bool)
    present[flat_t, flat_e] = True

    idx_lists = [np.nonzero(present[:, e])[0] for e in range(E)]
    max_count = max(len(ix) for ix in idx_lists)
    C = max(256, min(512, -(-max_count // 32) * 32))
    if max_count > 512:
        # Fall back: should not happen at T=2048/E=8/K=2, but keep correct —
        # split overflowing experts' tokens across multiple device passes.
        return _kernel_multipass(x, route, idx_lists, Wg, Wu, Wd)

    nc = _get_program(C)

    in_maps = []
    for e in range(E):
        ix = idx_lists[e]
        xT_e = np.zeros((H, C), np.float32)
        if len(ix):
            xT_e[:, : len(ix)] = x[ix].T
        in_maps.append({"xT": xT_e, "Wg": Wg[e], "Wu": Wu[e], "Wd": Wd[e]})

    res = run_bass_kernel_spmd(nc, in_maps, core_ids=list(range(E)))
    LAST_RESULTS = res

    out = np.zeros((T, H), np.float32)
    for e in range(E):
        ix = idx_lists[e]
        if len(ix) == 0:
            continue
        yT_e = res.results[e]["yT"]  # [H, C]
        out[ix] += route[ix, e][:, None] * yT_e[:, : len(ix)].T
    return out


def _kernel_multipass(x, route, idx_lists, Wg, Wu, Wd):
    """Correctness fallback if some expert holds > 512 tokens: run the
    device kernel several times, 512 tokens per expert per pass."""
    global LAST_RESULTS
    T = x.shape[0]
    out = np.zeros((T, H), np.float32)
    offs = [0] * E
    while any(offs[e] < len(idx_lists[e]) for e in range(E)):
        nc = _get_program(512)
        in_maps = []
        chunks = []
        for e in range(E):
            ix = idx_lists[e][offs[e] : offs[e] + 512]
            offs[e] += 512
            chunks.append(ix)
            xT_e = np.zeros((H, 512), np.float32)
            if len(ix):
                xT_e[:, : len(ix)] = x[ix].T
            in_maps.append({"xT": xT_e, "Wg": Wg[e], "Wu": Wu[e], "Wd": Wd[e]})
        res = run_bass_kernel_spmd(nc, in_maps, core_ids=list(range(E)))
        LAST_RESULTS = res
        for e in range(E):
            ix = chunks[e]
            if len(ix) == 0:
                continue
            yT_e = res.results[e]["yT"]
            out[ix] += route[ix, e][:, None] * yT_e[:, : len(ix)].T
    return out


# revision 7
# speedup vs baseline: 1.2136x; 1.2136x over previous
"""MoE (Gemma-style 8-expert top-2) Trainium2 kernel.

Strategy (expert-parallel over 8 NeuronCores):
  - Host: merge duplicate (token, expert) assignments, build per-expert token
    lists, gather+transpose x into xT_e [H, C] per expert (zero-padded to a
    common capacity C).  This is the "dispatch" all-to-all done host-side,
    which the full-input/full-output contract allows.  Weights are converted
    to fp16 and prepacked per 128-wide output tile so every device DMA is a
    simple 2D contiguous descriptor.
  - Device (per core e): dense expert MLP on its C tokens, all in transposed
    layout so every matmul uses natural weight layouts with zero on-device
    transposes:
        gateT[i, c] = sum_h Wg[h,i] * xT[h,c]     (weights stationary)
        upT   likewise
        hT    = gelu_tanh(gateT) * upT            [I, C]  (fp16 in SBUF)
        yT[h, c] = sum_i Wd[i,h] * hT[i,c]        [H, C]
    Matmul operands are fp16 (full PE rate, FWL weight loads); accumulation
    is fp32 in PSUM.  A short burst of dummy matmuls at kernel start warms
    the PE HAM clock-gate while the first DMAs land.
  - Host: combine — out[t] += route[t,e] * yT_e[:, pos].T  (the "combine"
    all-to-all), with route exactly matching the reference's scatter-add.
"""

import numpy as np

import concourse.bass as bass
import concourse.mybir as mybir
import concourse.tile as tile
from concourse import bacc


def _install_ntff_hook_shim():
    """The agent image's `antenv` lacks `axon_hooks`, which bass_utils
    imports unconditionally when tracing under axon.  Provide the module
    and register the ctypes-based NTFF profile hook so BASS_TRACE=1 yields
    real HW profiles.  Degrades silently if anything is missing."""
    import sys
    import types

    try:
        import antenv

        try:
            from antenv import axon_hooks  # noqa: F401

            return
        except ImportError:
            pass
        mod = types.ModuleType("antenv.axon_hooks")
        mod._hook = None
        mod.set_axon_ntff_profile_hook = lambda h: setattr(mod, "_hook", h)
        mod.get_axon_ntff_profile_hook = lambda: mod._hook
        sys.modules["antenv.axon_hooks"] = mod
        antenv.axon_hooks = mod
        import os

        so_path = "/opt/axon/libaxon_pjrt.so"
        if os.path.exists(so_path):
            from trn_agent_boot.trn_boot import _ntff_profile_via_ctypes

            mod._hook = _ntff_profile_via_ctypes(so_path)
    except Exception:
        pass


_install_ntff_hook_shim()

from concourse.bass_utils import run_bass_kernel_spmd

H = 2048
I = 4096
E = 8
P = 128
KH = H // P  # 16 contraction chunks for gate/up
MI = I // P  # 32 output tiles of I
KI = I // P  # 32 contraction chunks for down
MH = H // P  # 16 output tiles of H
F32 = mybir.dt.float32
F16 = mybir.dt.float16

# Results of the last device run (for test harnesses to inspect profiling).
LAST_RESULTS = None

_PROGRAM_CACHE: dict[int, "bass.Bass"] = {}


def _build_program(C: int) -> "bass.Bass":
    """Bass program for one core: expert MLP on C tokens (transposed layout)."""
    assert C % 32 == 0 and 256 <= C <= 512

    nc = bacc.Bacc("TRN2", target_bir_lowering=False)

    # Host-prepacked inputs: each [t, :, :] slab is one SBUF tile, contiguous.
    xT = nc.dram_tensor("xT", [H, C], F16, kind="ExternalInput")
    Wg = nc.dram_tensor("Wg", [MI, P, KH * P], F16, kind="ExternalInput")
    Wu = nc.dram_tensor("Wu", [MI, P, KH * P], F16, kind="ExternalInput")
    Wd = nc.dram_tensor("Wd", [MH, P, KI * P], F16, kind="ExternalInput")
    yT = nc.dram_tensor("yT", [H, C], F32, kind="ExternalOutput")

    xT_r = xT.rearrange("(k p) c -> p k c", p=P)  # [128, 16, C]
    yT_r = yT.rearrange("(m p) c -> p m c", p=P)  # [128, 16, C]
    Wg_a, Wu_a, Wd_a = Wg.ap(), Wu.ap(), Wd.ap()

    gelu = mybir.ActivationFunctionType.Gelu_apprx_tanh

    with tile.TileContext(nc) as tc:
        with (
            tc.tile_pool(name="xpool", bufs=1) as xpool,
            tc.tile_pool(name="hpool", bufs=1) as hpool,
            tc.tile_pool(name="wpool", bufs=3) as wpool,
            tc.tile_pool(name="tpool", bufs=3) as tpool,
            tc.tile_pool(name="warm", bufs=1) as warm_pool,
            tc.tile_pool(name="psum", bufs=2) as _psum_unused,  # keep name stable
            tc.tile_pool(name="psum2", bufs=2, space="PSUM") as psum_pool,
            tc.tile_pool(name="psumw", bufs=1, space="PSUM") as psum_warm,
        ):
            # --- PE warm-up: dummy matmuls over zeros while first DMAs land
            wz = warm_pool.tile([P, P], F16)
            xz = warm_pool.tile([P, C], F16)
            nc.vector.memset(wz, 0.0)
            nc.vector.memset(xz, 0.0)
            psum_w = psum_warm.tile([P, C], F32, tag="warm")
            for _ in range(16):
                nc.tensor.matmul(psum_w, wz, xz, start=True, stop=True)

            # x resident in SBUF: [128, 16, C] fp16
            xsb = xpool.tile([P, KH, C], F16)
            # first quarter early so m=0 matmuls can start ASAP
            nc.sync.dma_start(out=xsb[:, 0:4, :], in_=xT_r[:, 0:4, :])

            # h resident in SBUF: [128, 32, C] fp16
            hsb = hpool.tile([P, KI, C], F16)

            def load_w(dram_ap, t, tag):
                wt = wpool.tile([P, KH * P], F16, tag=tag, name=f"w_{tag}_{t}")
                nc.sync.dma_start(out=wt, in_=dram_ap[t])
                return wt.rearrange("p (k i) -> p k i", i=P)

            # ---- Phase 1: gateT/upT -> hT, one I-tile (128 rows) at a time
            for m in range(MI):
                wg_t = load_w(Wg_a, m, "wg")
                wu_t = load_w(Wu_a, m, "wu")
                if m == 0:
                    # rest of x arrives while m=0 computes
                    for q in range(1, 4):
                        nc.sync.dma_start(
                            out=xsb[:, 4 * q : 4 * (q + 1), :],
                            in_=xT_r[:, 4 * q : 4 * (q + 1), :],
                        )

                psum_g = psum_pool.tile([P, C], F32, tag="g")
                psum_u = psum_pool.tile([P, C], F32, tag="u")
                for k in range(KH):
                    nc.tensor.matmul(
                        psum_g,
                        wg_t[:, k, :],
                        xsb[:, k, :],
                        start=(k == 0),
                        stop=(k == KH - 1),
                    )
                for k in range(KH):
                    nc.tensor.matmul(
                        psum_u,
                        wu_t[:, k, :],
                        xsb[:, k, :],
                        start=(k == 0),
                        stop=(k == KH - 1),
                    )
                tg = tpool.tile([P, C], F32, tag="gelu")
                nc.scalar.activation(tg, psum_g, gelu)
                nc.vector.tensor_mul(hsb[:, m, :], tg, psum_u)

            # ---- Phase 2: downT -> yT, one H-tile (128 rows) at a time
            for m2 in range(MH):
                wd_t = wpool.tile([P, KI * P], F16, tag="wd", name=f"w_wd_{m2}")
                nc.sync.dma_start(out=wd_t, in_=Wd_a[m2])
                wd_v = wd_t.rearrange("p (k i) -> p k i", i=P)
                psum_d = psum_pool.tile([P, C], F32, tag="d")
                for k2 in range(KI):
                    nc.tensor.matmul(
                        psum_d,
                        wd_v[:, k2, :],
                        hsb[:, k2, :],
                        start=(k2 == 0),
                        stop=(k2 == KI - 1),
                    )
                ysb = tpool.tile([P, C], F32, tag="y")
                nc.vector.tensor_copy(ysb, psum_d)
                nc.sync.dma_start(out=yT_r[:, m2, :], in_=ysb)

    nc.compile()
    return nc


def _get_program(C: int) -> "bass.Bass":
    if C not in _PROGRAM_CACHE:
        _PROGRAM_CACHE[C] = _build_program(C)
    return _PROGRAM_CACHE[C]


def _prep_w_gu(w):  # [H, I] f32 -> [MI, P, KH*P] fp16, per-tile contiguous
    return np.ascontiguousarray(
        w.astype(np.float16).reshape(KH, P, MI, P).transpose(2, 1, 0, 3)
    ).reshape(MI, P, KH * P)


def _prep_w_d(w):  # [I, H] f32 -> [MH, P, KI*P] fp16
    return np.ascontiguousarray(
        w.astype(np.float16).reshape(KI, P, MH, P).transpose(2, 1, 0, 3)
    ).reshape(MH, P, KI * P)


def kernel(x, selected_experts, routing_weights, Wg, Wu, Wd):
    global LAST_RESULTS
    x = np.asarray(x, dtype=np.float32)
    se = np.asarray(selected_experts).astype(np.int64)
    rw = np.asarray(routing_weights).astype(np.float32)
    Wg = np.asarray(Wg, dtype=np.float32)
    Wu = np.asarray(Wu, dtype=np.float32)
    Wd = np.asarray(Wd, dtype=np.float32)

    T, K = se.shape
    assert x.shape == (T, H) and Wg.shape == (E, H, I) and Wd.shape == (E, I, H)

    # Dense route matrix, identical to the reference's scatter-add (merges
    # duplicate expert picks within a token by summing their weights).
    flat_t = np.repeat(np.arange(T), K)
    flat_e = se.ravel()
    route = np.zeros((T, E), np.float32)
    np.add.at(route, (flat_t, flat_e), rw.ravel())
    present = np.zeros((T, E), bool)
    present[flat_t, flat_e] = True

    idx_lists = [np.nonzero(present[:, e])[0] for e in range(E)]
    chunked = [
        [ix[s : s + 512] for s in range(0, max(len(ix), 1), 512)] for ix in idx_lists
    ]
    n_pass = max(len(ch) for ch in chunked)

    out = np.zeros((T, H), np.float32)
    for p in range(n_pass):
        parts = [ch[p] if p < len(ch) else np.empty(0, np.int64) for ch in chunked]
        max_count = max(len(ix) for ix in parts)
        C = max(256, min(512, -(-max(max_count, 1) // 32) * 32))
        nc = _get_program(C)
        in_maps = []
        for e in range(E):
            ix = parts[e]
            xT_e = np.zeros((H, C), np.float16)
            if len(ix):
                xT_e[:, : len(ix)] = x[ix].T.astype(np.float16)
            in_maps.append(
                {
                    "xT": xT_e,
                    "Wg": _prep_w_gu(Wg[e]),
                    "Wu": _prep_w_gu(Wu[e]),
                    "Wd": _prep_w_d(Wd[e]),
                }
            )
        res = run_bass_kernel_spmd(nc, in_maps, core_ids=list(range(E)))
        LAST_RESULTS = res
        for e in range(E):
            ix = parts[e]
            if len(ix) == 0:
                continue
            yT_e = res.results[e]["yT"]  # [H, C]
            out[ix] += route[ix, e][:, None] * yT_e[:, : len(ix)].T
    return out
